# revision 26
# baseline (speedup 1.0000x reference)
"""Trainium2 Bass kernel for nn_Attention (B=4, N=2048, C=768, H=8).

reference:
    qkv = x.reshape(B,N,H,d).transpose(0,2,1,3)      # q=k=v
    attn = softmax(q @ k^T / sqrt(d))
    out  = (attn @ v).transpose -> (B,N,C)
    y    = out @ proj_w.T + proj_b

Sharding: 8 cores = 4 batches x 2 query-halves. Each core holds all
keys/values of its batch and computes attention + the projection for its
1024 queries. No collectives; host concatenates the per-core outputs.

Layout: everything is computed transposed ([feature, token]) so that the
TensorEngine contraction dim is always the SBUF partition dim:
  S^T[k,q]   = (K_h^T)[d,ktile].T-contract @ (Q_h^T)[d,q]
  expS       = exp(scale*S^T)                      (ScalarE, PSUM->SBUF bf16)
  O^T[c,q]   = sum_kt [V_h | 1][ktile,97].T @ expS[ktile,q]   (row 96 = softmax denom)
  O_norm     = O^T * (1/denom)  (DVE recip + GpSimd partition_broadcast + DVE mul,
               then DMA-repacked into six 128-row c-chunks)
  Y^T[j,q]   = sum_c (W^T)[c_chunk, jtile].T @ O_norm[c_chunk, q] + bias

Schedule: software-pipelined stream over ktile groups (3,3,3,3,2,2); per
group the PE order is mm1(g) | mm2(g-1) | one projection-filler matmul,
so the ScalarE exp stream (the co-bottleneck with PE) is never starved.
The 2-ktile tail groups make the final exp of each head long enough to
cover the matmuls PE owes at the head boundary.  The projection of each
q-chunk is drained as filler during the next q-chunk's score phase.
PSUM: 2x3 banks score double-buffer + 1 PV accumulator + 1 projection
accumulator = 8 banks.
"""

import sys
import os

for _p in ("/opt/trn_rl_repo",):
    if os.path.isdir(_p) and _p not in sys.path:
        sys.path.insert(0, _p)

import numpy as np
import ml_dtypes

import concourse.bacc as bacc
import concourse.mybir as mybir
import concourse.tile as tile
from concourse.bass import MemorySpace
from concourse.bass_utils import run_bass_kernel_spmd

BF16 = ml_dtypes.bfloat16

B, N, C = 4, 2048, 768
H = 8
D = C // H            # 96
NCORES = 8
QPC = N // 2          # queries per core = 1024
QC = 512              # q chunk (PSUM free size)
NQC = QPC // QC       # 2
KT = N // 128         # 16 key tiles
JT = C // 128         # 6 output-feature tiles
SCALE = float(D) ** -0.5

_cache = {}


def build_bass(iters: int = 1):
    """Build the SPMD single-core program (same graph on all 8 cores)."""
    nc = bacc.Bacc("TRN2", target_bir_lowering=False, debug=False,
                   num_devices=NCORES)
    f32 = mybir.dt.float32
    bf16 = mybir.dt.bfloat16

    qt = nc.declare_dram_parameter("qt", [H, D, QPC], bf16, isOutput=False)
    kt = nc.declare_dram_parameter("kt", [H, D, N], bf16, isOutput=False)
    vn = nc.declare_dram_parameter("vn", [N, H, D + 1], bf16, isOutput=False)
    wt = nc.declare_dram_parameter("wt", [JT, 128, C], bf16, isOutput=False)
    bias = nc.declare_dram_parameter("bias", [JT, 128, 1], f32, isOutput=False)
    out = nc.declare_dram_parameter("out", [C, QPC], f32, isOutput=True)

    with tile.TileContext(nc) as tc:
        with (
            tc.tile_pool(name="consts", bufs=1) as consts,
            tc.tile_pool(name="expp", bufs=6) as expp,
            tc.tile_pool(name="small", bufs=8) as small,
            tc.tile_pool(name="onorm", bufs=2 * JT + 2) as onormp,
            tc.tile_pool(name="ysb", bufs=4) as ysbp,
            tc.tile_pool(name="ps_s", bufs=2, space=MemorySpace.PSUM) as ps_s,
            tc.tile_pool(name="ps_o", bufs=1, space=MemorySpace.PSUM) as ps_o,
            tc.tile_pool(name="ps_y", bufs=1, space=MemorySpace.PSUM) as ps_y,
        ):
            # ---- load constants (first-needed first) ----
            qt_sb = [consts.tile([D, QPC], bf16, tag=f"qt{h}", name=f"qt{h}")
                     for h in range(H)]
            kt_sb = [consts.tile([D, N], bf16, tag=f"kt{h}", name=f"kt{h}")
                     for h in range(H)]
            wt_sb = [consts.tile([128, C], bf16, tag=f"wt{c}", name=f"wt{c}")
                     for c in range(JT)]
            vn_sb = [consts.tile([128, H, D + 1], bf16, tag=f"vn{t_i}", name=f"vn{t_i}")
                     for t_i in range(KT)]
            bias_sb = [consts.tile([128, 1], f32, tag=f"bias{j}", name=f"bias{j}")
                       for j in range(JT)]
            # split first-needed loads so head 0 can start ASAP
            nc.sync.dma_start(out=qt_sb[0][:, 0:QC], in_=qt[0][:, 0:QC])
            nc.sync.dma_start(out=kt_sb[0][:, 0:QC], in_=kt[0][:, 0:QC])
            nc.sync.dma_start(out=qt_sb[0][:, QC:], in_=qt[0][:, QC:])
            nc.sync.dma_start(out=kt_sb[0][:, QC:], in_=kt[0][:, QC:])
            for t_i in range(3):
                nc.sync.dma_start(out=vn_sb[t_i][:],
                                  in_=vn[t_i * 128:(t_i + 1) * 128])
            for h in range(1, H):
                nc.sync.dma_start(out=qt_sb[h][:], in_=qt[h])
                nc.sync.dma_start(out=kt_sb[h][:], in_=kt[h])
                for t_i in range(3 + (h - 1) * 2, min(3 + h * 2, KT)):
                    nc.sync.dma_start(out=vn_sb[t_i][:],
                                      in_=vn[t_i * 128:(t_i + 1) * 128])
            for t_i in range(3 + (H - 1) * 2, KT):
                nc.sync.dma_start(out=vn_sb[t_i][:],
                                  in_=vn[t_i * 128:(t_i + 1) * 128])
            for c in range(JT):
                nc.sync.dma_start(out=wt_sb[c][:], in_=wt[c])
            for j in range(JT):
                nc.sync.dma_start(out=bias_sb[j][:], in_=bias[j])

            # HAM warmup: dummy matmuls with no input deps keep the PE
            # activity monitor busy during the initial DMA wait so real
            # matmuls start at full clock.
            wz = consts.tile([D, QC], bf16, tag="wz", name="wz")
            nc.vector.memset(wz[:], 0)
            pyw = ps_y.tile([128, QC], f32, tag="py", name="pyw")
            for _w in range(12):
                nc.tensor.matmul(pyw[:], wz[:, 0:128], wz[:],
                                 start=True, stop=True)
            # preload the ScalarE exp table set during the DMA wait
            wze = small.tile([1, 16], bf16, tag="wze", name="wze")
            nc.scalar.activation(out=wze[:], in_=wz[0:1, 0:16],
                                 func=mybir.ActivationFunctionType.Exp)

            # Software-pipelined stream.  Per 3-ktile group, PE program
            # order is: mm1(g) | mm2(g-1) | one projection-filler matmul;
            # ACT exp(g) follows mm1(g) immediately, so the next exp is
            # never queued behind mm2/projection work.
            # (3,3,3,3,2,2): the final 2-ktile exp (1038ns) covers the
            # 6 matmuls PE owes at a head boundary; a 1-ktile exp (612ns)
            # would leave the next head's first exp waiting on PE.
            groups = [[0, 1, 2], [3, 4, 5], [6, 7, 8], [9, 10, 11],
                      [12, 13], [14, 15]]
            from collections import deque

            pend = [None]        # deferred mm2 work: (po, es, grp, h, fin)
            projq = deque()      # projection units, one emitted per group

            def flush_pend():
                w = pend[0]
                if w is None:
                    return
                po, es, grp, h, fin = w
                pend[0] = None
                for i, t_i in enumerate(grp):
                    nc.tensor.matmul(
                        po[:],
                        vn_sb[t_i][:, h, :],
                        es[:, i, :],
                        start=(t_i == 0), stop=(t_i == KT - 1),
                    )
                if fin is not None:
                    fin()

            def emit_one_proj():
                if projq:
                    projq.popleft()()

            def emit_scores(qc, h, oglob):
                po = ps_o.tile([D + 1, QC], f32, tag="po")

                def normalize():
                    # copy PSUM->SBUF first so the accumulator bank frees
                    # immediately; the rest runs off the copy.
                    oc = small.tile([D + 1, QC], f32, tag="oc")
                    nc.vector.tensor_copy(out=oc[:], in_=po[:])
                    rc = small.tile([1, QC], f32, tag="rc")
                    nc.vector.reciprocal(out=rc[:], in_=oc[D:D + 1, :])
                    bc = small.tile([D, QC], f32, tag="bc")
                    nc.gpsimd.partition_broadcast(bc[:], rc[:])
                    on = small.tile([D, QC], bf16, tag="on")
                    nc.vector.tensor_mul(on[:], oc[0:D, :], bc[:])
                    # DMA-repack head rows 96h..96h+96 into the global
                    # 128-row c-chunk layout (DVE cannot shift partitions)
                    a0 = (D * h) % 128
                    c0 = (D * h) // 128
                    s1 = min(128 - a0, D)
                    nc.gpsimd.dma_start(out=oglob[c0][a0:a0 + s1, :],
                                        in_=on[0:s1, :])
                    if s1 < D:
                        nc.gpsimd.dma_start(out=oglob[c0 + 1][0:D - s1, :],
                                            in_=on[s1:D, :])

                for gi, grp in enumerate(groups):
                    ps = ps_s.tile([128, 3, QC], f32, tag="ps")
                    for i, t_i in enumerate(grp):
                        nc.tensor.matmul(
                            ps[:, i, :],
                            kt_sb[h][:, t_i * 128:(t_i + 1) * 128],
                            qt_sb[h][:, qc * QC:(qc + 1) * QC],
                            start=True, stop=True,
                        )
                    flush_pend()
                    emit_one_proj()
                    es = expp.tile([128, len(grp), QC], bf16, tag="es")
                    nc.scalar.activation(
                        out=es[:], in_=ps[:, 0:len(grp), :],
                        func=mybir.ActivationFunctionType.Exp,
                        scale=SCALE,
                    )
                    fin = normalize if gi == len(groups) - 1 else None
                    pend[0] = (po, es, grp, h, fin)

            def queue_proj(qc, oglob, final=False):
                py_box = [None]
                for j in range(JT):
                    def mk_mm(j, c):
                        def go():
                            if c == 0:
                                # at the tail the score PSUM slots are free:
                                # alternate accumulators so jtiles pipeline
                                if final and j % 2 == 1:
                                    pst = ps_s.tile([128, 3, QC], f32,
                                                    tag="ps", name="ps_t")
                                    py_box[0] = pst[:, 0, :]
                                else:
                                    py_box[0] = ps_y.tile([128, QC], f32,
                                                          tag="py", name="py")
                            nc.tensor.matmul(
                                py_box[0][:],
                                wt_sb[c][:, j * 128:(j + 1) * 128],
                                oglob[c][:],
                                start=(c == 0), stop=(c == JT - 1),
                            )
                        return go
                    for c in range(JT):
                        projq.append(mk_mm(j, c))

                    def mk_fin(j):
                        def go():
                            y = ysbp.tile([128, QC], f32, tag="y", name="y")
                            nc.vector.tensor_scalar_add(
                                out=y[:], in0=py_box[0][:],
                                scalar1=bias_sb[j][:],
                            )
                            nc.sync.dma_start(
                                out=out[j * 128:(j + 1) * 128,
                                        qc * QC:(qc + 1) * QC],
                                in_=y[:],
                            )
                        return go
                    projq.append(mk_fin(j))

            for it in range(iters):
                for qc in range(NQC):
                    oglob = [onormp.tile([128, QC], bf16, tag="og",
                                         name=f"og{qc}_{c}")
                             for c in range(JT)]
                    for h in range(H):
                        emit_scores(qc, h, oglob)
                    queue_proj(qc, oglob,
                               final=(it == iters - 1 and qc == NQC - 1))
            flush_pend()
            while projq:
                emit_one_proj()
    nc.compile()
    return nc


def shard_inputs(x, proj_w, proj_b):
    x = np.asarray(x, dtype=np.float32)
    proj_w = np.asarray(proj_w, dtype=np.float32)
    proj_b = np.asarray(proj_b, dtype=np.float32)

    wt_full = np.ascontiguousarray(proj_w.T).reshape(JT, 128, C).astype(BF16)
    bias_full = np.ascontiguousarray(proj_b).reshape(JT, 128, 1)

    in_maps = []
    for c in range(NCORES):
        b = c // 2
        q0 = (c % 2) * QPC
        xb = x[b]                                   # (N, C)
        xtb = np.ascontiguousarray(xb.T)            # (C, N)
        kt_c = xtb.reshape(H, D, N).astype(BF16)
        qt_c = np.ascontiguousarray(
            xtb[:, q0:q0 + QPC]).reshape(H, D, QPC).astype(BF16)
        vn_f = np.ones((N, H, D + 1), dtype=np.float32)
        vn_f[:, :, :D] = xb.reshape(N, H, D)
        in_maps.append({
            "qt": qt_c,
            "kt": kt_c,
            "vn": vn_f.astype(BF16),
            "wt": wt_full,
            "bias": bias_full,
        })
    return in_maps


def assemble(results):
    y = np.empty((B, N, C), dtype=np.float32)
    for c in range(NCORES):
        b = c // 2
        q0 = (c % 2) * QPC
        y[b, q0:q0 + QPC, :] = results[c]["out"].T
    return y


def kernel(x, proj_w, proj_b):
    if "nc" not in _cache:
        _cache["nc"] = build_bass()
    nc = _cache["nc"]
    in_maps = shard_inputs(x, proj_w, proj_b)
    res = run_bass_kernel_spmd(nc, in_maps, core_ids=list(range(NCORES)))
    return assemble(res.results)
